# revision 2
# baseline (speedup 1.0000x reference)
"""DSimilarity.gradgrad force-force covariance block on 8 Trainium2 cores.

out[m*3+a, n*3+b] = sum_{i,j} u1[i,a]*u2[j,b]*gg[i,j]*[i1[i]==m]*[i2[j]==n]
with gg[i,j] = (c - c^2 diff^2) * exp(-0.5 c diff^2), diff = d1[i]-d2[j], c=1/l^2.

gg depends only on the scalar difference d1[i]-d2[j], so the 4000x4000 kernel
matrix separates: a 2D Chebyshev expansion of gg on the observed d-range,
truncated by SVD, gives gg ~= sum_k phi_k(d1) psi_k(d2) with rank ~16 at
machine precision (rank 32 used for margin). Folding the scatter matrices in
on the host reduces the whole computation to out = M^T @ W with
M[k, 3m+a] = sum_{i1[i]=m} phi_k(d1_i) u1[i,a]   (shared by all cores)
W[k, 3n+b] = sum_{i2[j]=n} psi_k(d2_j) u2[j,b]   (column strip per core).

Each core computes a 62-63-atom strip of output columns: 12 rank-32 matmuls
[32,128]^T @ [32,256] (one per 128-row block), PSUM->SBUF copies alternating
DVE/ACT, and two ~0.58MB output DMAs split across the two HWDGE rings.
"""

import math
import sys
import types

import numpy as np

NCORES = 8
CHEB_K = 64  # chebyshev grid size for the 2D expansion

TRACE = False  # test.py sets True to capture an NTFF profile
LAST_RESULTS = None  # BassKernelResults of the last run (for test.py)

_PROGRAM_CACHE = {}


def _install_ntff_hook():
    try:
        from antenv.axon_hooks import get_axon_ntff_profile_hook  # noqa: F401
        return
    except ImportError:
        pass
    try:
        from trn_agent_boot.trn_boot import _ntff_profile_via_ctypes
        import antenv
        hook = _ntff_profile_via_ctypes('/opt/axon/libaxon_pjrt.so')
        mod = types.ModuleType("antenv.axon_hooks")
        mod._hook = hook
        mod.get_axon_ntff_profile_hook = lambda: mod._hook
        mod.set_axon_ntff_profile_hook = lambda h: setattr(mod, "_hook", h)
        antenv.axon_hooks = mod
        sys.modules["antenv.axon_hooks"] = mod
    except Exception:
        pass


def _build_program(R, W3P, W3, NBLK):
    """Per-core Bass program (same on all 8 cores): out = M^T @ W.

    dram input "mw" = [W (W3P cols) | M (NBLK*128 cols)] so one DMA covers
    W + the first M block and the rest streams behind it on the other ring.
    """
    import concourse.bacc as bacc
    import concourse.tile as tile
    import concourse.mybir as mybir

    F32 = mybir.dt.float32
    F32R = mybir.dt.float32r

    nc = bacc.Bacc("TRN2", target_bir_lowering=False, debug=False)
    mw_h = nc.dram_tensor("mw", [R, W3P + NBLK * 128], F32R,
                          kind="ExternalInput")
    out_h = nc.dram_tensor("out", [128, NBLK * W3], F32, kind="ExternalOutput")

    NB1 = NBLK // 2

    with tile.TileContext(nc) as tc:
        with (
            tc.tile_pool(name="const", bufs=1) as cpool,
            tc.tile_pool(name="ps", bufs=4, space="PSUM") as hpool,
        ):
            mw = cpool.tile([R, W3P + NBLK * 128], F32R)
            # head DMA: W + M block 0 (gates matmul 0); tail rides ACT ring
            head = W3P + 128
            nc.sync.dma_start(out=mw[:, :head], in_=mw_h[:, :head])
            nc.scalar.dma_start(out=mw[:, head:], in_=mw_h[:, head:])
            o_stage = cpool.tile([128, NBLK, W3], F32)
            for blk in range(NBLK):
                ps = hpool.tile([128, W3P], F32, tag="ps")
                nc.tensor.matmul(
                    ps[:, :],
                    mw[:, W3P + blk * 128:W3P + (blk + 1) * 128],
                    mw[:, :W3P],
                    start=True, stop=True)
                if blk % 2 == 0:
                    nc.vector.tensor_copy(o_stage[:, blk, :], ps[:, :W3])
                else:
                    nc.scalar.copy(o_stage[:, blk, :], ps[:, :W3])
                if blk == NB1 - 1:
                    nc.sync.dma_start(out=out_h[:, :NB1 * W3],
                                      in_=o_stage[:, :NB1, :])
                elif blk == NBLK - 1:
                    nc.scalar.dma_start(out=out_h[:, NB1 * W3:],
                                        in_=o_stage[:, NB1:, :])
    nc.compile()
    return nc


def _cheb_factors(d1, d2, c, R):
    """Rank-R separation gg(d1_i - d2_j) ~= Phi[:, i]^T Psi[:, j]."""
    lo = min(d1.min(), d2.min())
    hi = max(d1.max(), d2.max())
    mid = 0.5 * (lo + hi)
    half = 0.5 * (hi - lo) * 1.0000001 + 1e-12

    K = CHEB_K
    m = np.arange(K)
    xg = np.cos(np.pi * (m + 0.5) / K)  # chebyshev roots grid

    def gg_fn(diff):
        e = np.exp(-0.5 * c * diff * diff)
        return (c - diff * diff * c * c) * e

    F = gg_fn(half * (xg[:, None] - xg[None, :]))
    T = np.cos(np.pi * np.outer(m + 0.5, m) / K)  # T[m, p] = T_p(x_m)
    C = (2.0 / K) ** 2 * (T.T @ F @ T)
    C[0, :] *= 0.5
    C[:, 0] *= 0.5
    U, S, Vt = np.linalg.svd(C)
    r = int(min(R, K))
    cu = U[:, :r] * np.sqrt(S[:r])
    cv = Vt[:r].T * np.sqrt(S[:r])
    Phi = np.polynomial.chebyshev.chebval((d1 - mid) / half, cu)  # [r, n1]
    Psi = np.polynomial.chebyshev.chebval((d2 - mid) / half, cv)  # [r, n2]
    if r < R:
        Phi = np.concatenate([Phi, np.zeros((R - r, d1.size))], axis=0)
        Psi = np.concatenate([Psi, np.zeros((R - r, d2.size))], axis=0)
    return Phi, Psi


def kernel(**inputs):
    global LAST_RESULTS
    d1 = np.asarray(inputs["d1"], dtype=np.float64).reshape(-1)
    u1 = np.asarray(inputs["u1"], dtype=np.float64)
    d2 = np.asarray(inputs["d2"], dtype=np.float64).reshape(-1)
    u2 = np.asarray(inputs["u2"], dtype=np.float64)
    ls = float(np.asarray(inputs["lengthscale"]).reshape(-1)[0])
    i1 = np.asarray(inputs["i1"]).reshape(-1).astype(np.int64)
    i2 = np.asarray(inputs["i2"]).reshape(-1).astype(np.int64)
    na1 = int(np.asarray(inputs["natoms1"]))
    na2 = int(np.asarray(inputs["natoms2"]))
    n1 = d1.shape[0]
    c = 1.0 / (ls * ls)

    R = 32
    Phi, Psi = _cheb_factors(d1, d2, c, R)

    # fold u1 + segment-sum over i1 into the shared row factor M [R, 3*na1]
    NROW = 3 * na1
    NBLK = (NROW + 127) // 128
    Mt = np.zeros((NBLK * 128, R))
    for a in range(3):
        np.add.at(Mt, 3 * i1 + a, (Phi * u1[:, a]).T)
    M = np.ascontiguousarray(Mt.T, dtype=np.float32)  # [R, NBLK*128]

    # fold u2 + segment-sum over i2 into the column factor W [R, na2, 3]
    Wt = np.zeros((3 * na2, R))
    for b in range(3):
        np.add.at(Wt, 3 * i2 + b, (Psi * u2[:, b]).T)
    W_full = Wt.T.reshape(R, na2, 3)

    # column strips: first (na2 % NCORES) cores get one extra atom
    base = na2 // NCORES
    rem = na2 % NCORES
    counts = [base + (1 if cc < rem else 0) for cc in range(NCORES)]
    starts = np.concatenate([[0], np.cumsum(counts)]).astype(np.int64)
    wmax = max(counts)
    W3 = 3 * wmax
    W3P = ((W3 + 255) // 256) * 256

    key = (R, W3P, W3, NBLK)
    nc = _PROGRAM_CACHE.get(key)
    if nc is None:
        nc = _build_program(R, W3P, W3, NBLK)
        _PROGRAM_CACHE[key] = nc

    in_maps = []
    for cc in range(NCORES):
        a0, a1 = int(starts[cc]), int(starts[cc + 1])
        mw = np.zeros((R, W3P + NBLK * 128), np.float32)
        mw[:, :3 * (a1 - a0)] = W_full[:, a0:a1, :].reshape(R, -1)
        mw[:, W3P:] = M
        in_maps.append({"mw": mw})

    from concourse.bass_utils import run_bass_kernel_spmd
    if TRACE:
        _install_ntff_hook()
    res = run_bass_kernel_spmd(nc, in_maps, core_ids=list(range(NCORES)),
                               trace=TRACE)
    LAST_RESULTS = res

    out = np.zeros((3 * na1, 3 * na2), np.float32)
    for cc in range(NCORES):
        a0, a1 = int(starts[cc]), int(starts[cc + 1])
        w3 = 3 * (a1 - a0)
        part = res.results[cc]["out"].reshape(128, NBLK, W3)
        part = part.transpose(1, 0, 2).reshape(NBLK * 128, W3)
        out[:, 3 * a0:3 * a0 + w3] = part[:NROW, :w3]
    return out
